# revision 13
# baseline (speedup 1.0000x reference)
"""Trainium2 Bass kernel for masked single-head attention.

Reference computation (per batch b):
    Q = q_hidden[b] @ Wq + bq            # [S, D]
    K = k_hidden[b] @ Wk + bk            # [S, D]
    V = v_hidden[b] @ Wv + bv            # [S, D]
    S_qk = (Q @ K.T) / sqrt(D)           # [S, S]
    S_qk = where(mask[b]==0, -1e9, S_qk)
    out[b] = softmax(S_qk, -1) @ V       # [S, D]

Sharding: data-parallel over batch, one batch per NeuronCore (B == 8 cores).
No collectives.

Device-side dataflow (per core, S=2048, HID=1024, D=64):
  - host ships transposed hiddens qT/kT/vT [HID, S] fp16, the mask
    transposed as {0,1} fp8, Wq pre-scaled by 1/sqrt(D).  (fp8 hiddens /
    probabilities were tried and fail the 2e-2 gate: random-sign sums do
    not shrink relative quantization error, and the scale-relative MAX
    metric sits ~4.5 sigma out; measured 2-9e-2 per fp8 tensor.)
  - projections: fp16 matmuls column-packed into PSUM partition groups
    0-63 / 64-127 (concurrent, so the pair costs one stream).  Q is
    projected into both groups (score row packing needs QT duplicated).
  - scores: fp16 QK^T row-packed pairs (kta on array rows 0-63, ktb=kta+4
    on rows 64-127) into one [128, 2, 512] PSUM tile; exp on ScalarE
    emits fp16; the mask is applied post-exp as a multiply on VectorE
    (masked lanes become exactly 0, no -1e9 offset needed since
    unmasked scores are < 3).
  - out^T[65, q] accumulates over k tiles: lhsT=[V_kt|1] [128, 65];
    row 64 is the softmax denominator.
  - norm: PE-transpose [65,128] slices back to [128,65], reciprocal of
    the per-partition denominator column (cheap: [128,1] layout),
    multiply on GpSimd, DMA out [q, 64].
  - engine split: ScalarE = exps only (plus early q DMA triggers);
    VectorE = K copies + mask multiplies + reciprocals; GpSimd = Q/V
    copies, norm copies/muls, mask+k DMA triggers; SyncE = bulk DMA.
"""

import numpy as np

import concourse.bass as bass
import concourse.tile as tile
from concourse import bacc
from concourse import mybir
from concourse.bass_utils import run_bass_kernel_spmd

B, S, HID, D = 8, 2048, 1024, 64
NCORES = 8
HCH = HID // 128          # 8 hidden chunks
KT_TILES = S // 128       # 16 k tiles
NPAIR = KT_TILES // 2     # 8 k-tile pairs (kta, kta+4)
NQ = 512                  # q chunk width for the attention inner loop
QCH = S // NQ             # 4

F32 = mybir.dt.float32
F16 = mybir.dt.float16
FP8 = mybir.dt.float8e4

LAST_EXEC_TIME_NS = None
_CACHED = {}


def _build_program(with_qk_bias=False):
    nc = bacc.Bacc("TRN2", target_bir_lowering=False, debug=False,
                   num_swdge_queues=4)

    qT_d = nc.dram_tensor("qT", [HID, S], F16, kind="ExternalInput").ap()
    kT_d = nc.dram_tensor("kT", [HID, S], F16, kind="ExternalInput").ap()
    vT_d = nc.dram_tensor("vT", [HID, S], F16, kind="ExternalInput").ap()
    maskT_d = nc.dram_tensor("maskT", [S, S], FP8, kind="ExternalInput").ap()
    wq_d = nc.dram_tensor("wq", [HID, D], F16, kind="ExternalInput").ap()
    wk_d = nc.dram_tensor("wk", [HID, D], F16, kind="ExternalInput").ap()
    wv_d = nc.dram_tensor("wv", [HID, D], F16, kind="ExternalInput").ap()
    if with_qk_bias:
        bq_d = nc.dram_tensor("bq", [D], F32, kind="ExternalInput").ap()
        bk_d = nc.dram_tensor("bk", [D], F32, kind="ExternalInput").ap()
    idf_d = nc.dram_tensor("idf", [128, 128], F32, kind="ExternalInput").ap()
    out_d = nc.dram_tensor("out", [S, D], F32, kind="ExternalOutput").ap()

    ExpF = mybir.ActivationFunctionType.Exp

    qT_r = qT_d.rearrange("(o p) s -> p o s", p=128)   # [128, HCH, S]
    kT_r = kT_d.rearrange("(o p) s -> p o s", p=128)
    vT_r = vT_d.rearrange("(o p) s -> p o s", p=128)

    def _body(tc):
        with tc.tile_pool(name="const", bufs=1) as const:
            w_q = const.tile([128, HCH, D], F16, name="w_q")
            w_k = const.tile([128, HCH, D], F16, name="w_k")
            w_v = const.tile([128, HCH, D], F16, name="w_v")
            idf = const.tile([128, 128], F32, name="idf")
            idf16 = const.tile([128, 128], F16, name="idf16")

            # masksb pair-major: [p, pair, slot, q] with k-tile = 8g+4s+i,
            # pair index 4g+i, slot s.
            masksb = const.tile([128, NPAIR, 2, S], FP8, name="masksb")
            qh = const.tile([128, HCH, S], F16, name="qh")
            kh = const.tile([128, HCH, S], F16, name="kh")
            vh = const.tile([128, HCH, S], F16, name="vh")

            # ---- DMA issue order == consumption order -------------------
            # scalar (ACT): first two q chunks only, then ACT is all exps.
            for c in range(2):
                cs = slice(c * NQ, (c + 1) * NQ)
                for h in range(0, HCH, 2):
                    nc.scalar.dma_start(qh[:, h:h + 2, cs],
                                        qT_r[:, h:h + 2, cs])

            # sync: weights, k col-half0 (h0-3), k col-half1 (h0-3),
            # q col-half1 (h0-3), v (h0-3).
            nc.sync.dma_start(w_q, wq_d.rearrange("(o p) d -> p o d", p=128))
            nc.sync.dma_start(w_k, wk_d.rearrange("(o p) d -> p o d", p=128))
            nc.sync.dma_start(w_v, wv_d.rearrange("(o p) d -> p o d", p=128))
            nc.sync.dma_start(idf, idf_d)
            if with_qk_bias:
                b_q = const.tile([128, 1], F32, name="b_q")
                b_k = const.tile([128, 1], F32, name="b_k")
                nc.sync.dma_start(b_q[0:D, :], bq_d.unsqueeze(1))
                nc.sync.dma_start(b_q[64:64 + D, :], bq_d.unsqueeze(1))
                nc.sync.dma_start(b_k[0:D, :], bk_d.unsqueeze(1))
                nc.sync.dma_start(b_k[64:64 + D, :], bk_d.unsqueeze(1))
            else:
                b_q = b_k = None
            cs0 = slice(0, 1024)
            cs1 = slice(1024, 2048)
            for csl in (cs0, cs1):
                for h in range(0, 4, 2):
                    nc.sync.dma_start(kh[:, h:h + 2, csl],
                                      kT_r[:, h:h + 2, csl])
            for h in range(0, 4, 2):
                nc.sync.dma_start(qh[:, h:h + 2, cs1], qT_r[:, h:h + 2, cs1])
            for h in range(0, 4, 2):
                nc.sync.dma_start(vh[:, h:h + 2, :], vT_r[:, h:h + 2, :])

            # gpsimd: k col-halves (h4-7), q col-half1 (h4-7), masks
            # (pair-group major), v (h4-7).
            for csl in (cs0, cs1):
                for h in range(4, HCH, 2):
                    nc.gpsimd.dma_start(kh[:, h:h + 2, csl],
                                        kT_r[:, h:h + 2, csl])
            for h in range(4, HCH, 2):
                nc.gpsimd.dma_start(qh[:, h:h + 2, cs1],
                                    qT_r[:, h:h + 2, cs1])
            for g in range(2):
                for s_ in range(2):
                    r0 = (8 * g + 4 * s_) * 128
                    nc.gpsimd.dma_start(
                        masksb[:, 4 * g:4 * (g + 1), s_, :],
                        maskT_d[r0:r0 + 4 * 128, :].rearrange(
                            "(i p) q -> p i q", p=128))
            for h in range(4, HCH, 2):
                nc.gpsimd.dma_start(vh[:, h:h + 2, :], vT_r[:, h:h + 2, :])

            nc.vector.tensor_copy(idf16, idf)

            QT = const.tile([128, S], F16, name="QT")
            KT = const.tile([128, S], F16, name="KT")
            VT = const.tile([128, S], F16, name="VT")
            Vt = const.tile([128, KT_TILES, D + 1], F16, name="Vt")
            nc.vector.memset(Vt[:, :, D:D + 1], 1.0)

            with tc.tile_pool(name="stp", bufs=2, space="PSUM") as stp, \
                 tc.tile_pool(name="ntp", bufs=2, space="PSUM") as ntp, \
                 tc.tile_pool(name="ptp", bufs=26) as ptp, \
                 tc.tile_pool(name="nsb", bufs=2) as nsb:

                def q_proj(c):
                    # both column groups get the same data (QT rows 64-127
                    # duplicate rows 0-63 for score row packing); the two
                    # groups run concurrently so the dup is free.
                    cs = slice(c * NQ, (c + 1) * NQ)
                    prja = stp.tile([128, NQ], F32, name="prja", tag="prj",
                                    bufs=2)
                    prjb = stp.tile([128, NQ], F32, name="prjb", tag="prj",
                                    bufs=2)
                    for h in range(HCH):
                        st_, sp_ = (h == 0), (h == HCH - 1)
                        nc.tensor.matmul(
                            prja[0:D, :], lhsT=w_q[:, h, :],
                            rhs=qh[:, h, cs], start=st_, stop=sp_)
                        nc.tensor.matmul(
                            prjb[64:64 + D, :], lhsT=w_q[:, h, :],
                            rhs=qh[:, h, cs], start=st_, stop=sp_)
                    nc.vector.tensor_copy(QT[0:D, cs], prja[0:D, :])
                    nc.vector.tensor_copy(QT[64:128, cs], prjb[64:128, :])
                    if b_q is not None:
                        nc.vector.tensor_scalar_add(QT[:, cs], QT[:, cs], b_q)

                def kv_proj_cp(hid_t, w_t, b_t, dest, cp, eng):
                    # one column-packed pair of 512-chunks: ca -> rows 0-63,
                    # cb -> rows 64-127.
                    ca = slice((2 * cp) * 512, (2 * cp + 1) * 512)
                    cb = slice((2 * cp + 1) * 512, (2 * cp + 2) * 512)
                    prja = stp.tile([128, 512], F32, name="prja", tag="prj",
                                    bufs=2)
                    prjb = stp.tile([128, 512], F32, name="prjb", tag="prj",
                                    bufs=2)
                    for h in range(HCH):
                        st_, sp_ = (h == 0), (h == HCH - 1)
                        nc.tensor.matmul(
                            prja[0:D, :], lhsT=w_t[:, h, :],
                            rhs=hid_t[:, h, ca], start=st_, stop=sp_)
                        nc.tensor.matmul(
                            prjb[64:64 + D, :], lhsT=w_t[:, h, :],
                            rhs=hid_t[:, h, cb], start=st_, stop=sp_)
                    eng.tensor_copy(dest[0:D, ca], prja[0:D, :])
                    eng.tensor_copy(dest[64:128, cb], prjb[64:128, :])
                    if b_t is not None:
                        eng.tensor_scalar_add(
                            dest[0:D, ca], dest[0:D, ca], b_t[0:D, :])
                        eng.tensor_scalar_add(
                            dest[64:64 + D, cb], dest[64:64 + D, cb],
                            b_t[64:64 + D, :])

                def v_finish():
                    # V^T -> Vt[p, kt, D+1]; odd 512-chunks of VT live on
                    # rows 64-127 (column packing).
                    for kt in range(KT_TILES):
                        rb = 0 if (kt // 4) % 2 == 0 else 64
                        vtr = ntp.tile([128, D], F16, name="vtr", tag="tr")
                        nc.tensor.transpose(
                            vtr, VT[rb:rb + D, kt * 128:(kt + 1) * 128],
                            idf16[rb:rb + D, rb:rb + D])
                        nc.vector.tensor_copy(Vt[:, kt, :D], vtr)

                def sc_exp(qc, p):
                    # row-packed fp16 score pair (kta rows 0-63, ktb rows
                    # 64-127), exp -> fp16.
                    q0 = qc * NQ
                    qsl = slice(q0, q0 + NQ)
                    g, i = divmod(p, 4)
                    kta, ktb = 8 * g + i, 8 * g + i + 4
                    sa = slice(kta * 128, kta * 128 + 128)
                    sb = slice(ktb * 128, ktb * 128 + 128)
                    st = stp.tile([128, 2, NQ], F32, name="st", tag="st")
                    nc.tensor.matmul(
                        st[:, 0, :], lhsT=KT[0:D, sa], rhs=QT[0:D, qsl],
                        start=True, stop=True)
                    nc.tensor.matmul(
                        st[:, 1, :], lhsT=KT[64:64 + D, sb],
                        rhs=QT[64:64 + D, qsl],
                        start=True, stop=True)
                    pt = ptp.tile([128, 2, NQ], F16, name="pt", tag="pt")
                    nc.scalar.activation(pt, st, ExpF)
                    return pt

                def mask_mul(qc, p, pt):
                    q0 = qc * NQ
                    qsl = slice(q0, q0 + NQ)
                    eng = nc.vector if (qc + p) % 2 == 0 else nc.gpsimd
                    eng.tensor_mul(pt, pt, masksb[:, p, :, qsl])

                def av(outT, p, pt, first, last):
                    g, i = divmod(p, 4)
                    kta, ktb = 8 * g + i, 8 * g + i + 4
                    nc.tensor.matmul(
                        outT, lhsT=Vt[:, kta, :], rhs=pt[:, 0, :],
                        start=first, stop=False)
                    nc.tensor.matmul(
                        outT, lhsT=Vt[:, ktb, :], rhs=pt[:, 1, :],
                        start=False, stop=last)

                def norm(qc, outT):
                    q0 = qc * NQ
                    outT_sb = nsb.tile([D + 1, NQ], F32, name="outT_sb",
                                       tag="outT_sb")
                    nc.vector.tensor_copy(outT_sb, outT)
                    o_big = nsb.tile([128, NQ // 128, D], F32, name="o_big",
                                     tag="o_big")
                    for i in range(NQ // 128):
                        tr = ntp.tile([128, D + 1], F32, name="tr", tag="tr")
                        nc.tensor.transpose(
                            tr, outT_sb[:, i * 128:(i + 1) * 128],
                            idf[:D + 1, :D + 1])
                        tr_sb = nsb.tile([128, D + 1], F32, name="tr_sb",
                                         tag="tr_sb")
                        nc.vector.tensor_copy(tr_sb, tr)
                        nc.vector.reciprocal(tr_sb[:, D:D + 1],
                                             tr_sb[:, D:D + 1])
                        nc.gpsimd.tensor_scalar_mul(
                            o_big[:, i, :], tr_sb[:, :D], tr_sb[:, D:D + 1])
                    nc.sync.dma_start(
                        out_d[q0:q0 + NQ, :].rearrange("(t p) d -> p t d",
                                                       p=128), o_big)

                # ---- staged emission (PE stream order == data arrival) --
                # wave 0 = pairs 0-3 (k-tiles 0-7: K chunk-pair 0, mask
                # g=0 blocks); wave 1 = pairs 4-7 (k-tiles 8-15).
                pts = {}
                kv_proj_cp(kh, w_k, b_k, KT, 0, nc.vector)
                for qc in range(QCH):
                    q_proj(qc)
                    for p in range(4):
                        pts[(qc, p)] = sc_exp(qc, p)
                        mask_mul(qc, p, pts[(qc, p)])
                kv_proj_cp(vh, w_v, None, VT, 0, nc.vector)
                kv_proj_cp(vh, w_v, None, VT, 1, nc.vector)
                v_finish()
                kv_proj_cp(kh, w_k, b_k, KT, 1, nc.vector)
                for qc in range(QCH):
                    for p in range(4, NPAIR):
                        pts[(qc, p)] = sc_exp(qc, p)
                        mask_mul(qc, p, pts[(qc, p)])
                    outT = stp.tile([D + 1, NQ], F32, name="outT",
                                    tag="prj", bufs=2)
                    for p in range(NPAIR):
                        av(outT, p, pts[(qc, p)], p == 0, p == NPAIR - 1)
                    norm(qc, outT)

    with tile.TileContext(nc) as tc:
        _body(tc)

    nc.compile()
    return nc


def _prep_inputs(q_hidden_inputs, k_hidden_inputs, v_hidden_inputs, mask,
                 Wq, bq, Wk, bk, Wv, bv):
    scale = np.float32(1.0 / np.sqrt(np.float32(D)))
    wq = (np.asarray(Wq, np.float32) * scale).astype(np.float16)
    wk = np.asarray(Wk, np.float32).astype(np.float16)
    wv = np.asarray(Wv, np.float32).astype(np.float16)
    bqs = np.asarray(bq, np.float32) * scale
    bks = np.asarray(bk, np.float32)
    with_qk_bias = bool(np.any(bqs != 0) or np.any(bks != 0))
    idf = np.eye(128, dtype=np.float32)

    q = np.asarray(q_hidden_inputs, np.float32)
    k = np.asarray(k_hidden_inputs, np.float32)
    v = np.asarray(v_hidden_inputs, np.float32)
    m = np.asarray(mask)

    import ml_dtypes
    FP8_NP = ml_dtypes.float8_e4m3

    in_maps = []
    for b in range(B):
        im = {
            "qT": np.ascontiguousarray(q[b].T).astype(np.float16),
            "kT": np.ascontiguousarray(k[b].T).astype(np.float16),
            "vT": np.ascontiguousarray(v[b].T).astype(np.float16),
            "maskT": (np.ascontiguousarray(m[b].T) != 0).astype(
                np.float32).astype(FP8_NP),
            "wq": wq, "wk": wk, "wv": wv,
            "idf": idf,
        }
        if with_qk_bias:
            im["bq"] = bqs
            im["bk"] = bks
        in_maps.append(im)
    return in_maps, with_qk_bias


def kernel(q_hidden_inputs, k_hidden_inputs, v_hidden_inputs, mask,
           Wq, bq, Wk, bk, Wv, bv, trace=False):
    global LAST_EXEC_TIME_NS
    in_maps, with_qk_bias = _prep_inputs(
        q_hidden_inputs, k_hidden_inputs, v_hidden_inputs,
        mask, Wq, bq, Wk, bk, Wv, bv)
    key = ("nc", with_qk_bias)
    if key not in _CACHED:
        _CACHED[key] = _build_program(with_qk_bias)
    nc = _CACHED[key]

    res = run_bass_kernel_spmd(nc, in_maps, list(range(NCORES)), trace=trace)
    LAST_EXEC_TIME_NS = res.exec_time_ns
    out = np.stack([res.results[b]["out"] for b in range(B)], axis=0)
    # bv folds into the output exactly: softmax rows sum to 1, so
    # attn @ (V + 1 bv^T) = attn @ V + bv.
    out = out + np.asarray(bv, np.float32)[None, None, :]
    return out


# revision 15
# speedup vs baseline: 1.1445x; 1.1445x over previous
"""Trainium2 Bass kernel for masked single-head attention.

Reference computation (per batch b):
    Q = q_hidden[b] @ Wq + bq            # [S, D]
    K = k_hidden[b] @ Wk + bk            # [S, D]
    V = v_hidden[b] @ Wv + bv            # [S, D]
    S_qk = (Q @ K.T) / sqrt(D)           # [S, S]
    S_qk = where(mask[b]==0, -1e9, S_qk)
    out[b] = softmax(S_qk, -1) @ V       # [S, D]

Sharding: data-parallel over batch, one batch per NeuronCore (B == 8 cores).
No collectives.

Device-side dataflow (per core, S=2048, HID=1024, D=64):
  - host ships transposed hiddens qT/kT/vT [HID, S] fp16, the mask
    transposed as {0,1} fp8, Wq pre-scaled by 1/sqrt(D).  (fp8 hiddens /
    probabilities were tried and fail the 2e-2 gate: random-sign sums do
    not shrink relative quantization error, and the scale-relative MAX
    metric sits ~4.5 sigma out; measured 2-9e-2 per fp8 tensor.)
  - projections: fp16 matmuls column-packed into PSUM partition groups
    0-63 / 64-127 (concurrent, so the pair costs one stream).  Q is
    projected into both groups (score row packing needs QT duplicated).
  - scores: fp16 QK^T row-packed pairs (kta on array rows 0-63, ktb=kta+4
    on rows 64-127) into one [128, 2, 512] PSUM tile; exp on ScalarE
    emits fp16; the mask is applied post-exp as a multiply on VectorE
    (masked lanes become exactly 0, no -1e9 offset needed since
    unmasked scores are < 3).
  - out^T[65, q] accumulates over k tiles: lhsT=[V_kt|1] [128, 65];
    row 64 is the softmax denominator.
  - norm: PE-transpose [65,128] slices back to [128,65], reciprocal of
    the per-partition denominator column (cheap: [128,1] layout),
    multiply on GpSimd, DMA out [q, 64].
  - engine split: ScalarE = exps only (plus early q DMA triggers);
    VectorE = K copies + mask multiplies + reciprocals; GpSimd = Q/V
    copies, norm copies/muls, mask+k DMA triggers; SyncE = bulk DMA.
"""

import numpy as np

import concourse.bass as bass
import concourse.tile as tile
from concourse import bacc
from concourse import mybir
from concourse.bass_utils import run_bass_kernel_spmd

B, S, HID, D = 8, 2048, 1024, 64
NCORES = 8
HCH = HID // 128          # 8 hidden chunks
KT_TILES = S // 128       # 16 k tiles
NPAIR = KT_TILES // 2     # 8 k-tile pairs (kta, kta+4)
NQ = 512                  # q chunk width for the attention inner loop
QCH = S // NQ             # 4

F32 = mybir.dt.float32
F16 = mybir.dt.float16
FP8 = mybir.dt.float8e4

LAST_EXEC_TIME_NS = None
_CACHED = {}


def _build_program(with_qk_bias=False):
    nc = bacc.Bacc("TRN2", target_bir_lowering=False, debug=False,
                   num_swdge_queues=4)

    qT_d = nc.dram_tensor("qT", [HID, S], F16, kind="ExternalInput").ap()
    kT_d = nc.dram_tensor("kT", [HID, S], F16, kind="ExternalInput").ap()
    vT_d = nc.dram_tensor("vT", [HID, S], F16, kind="ExternalInput").ap()
    maskT_d = nc.dram_tensor("maskT", [S, S], FP8, kind="ExternalInput").ap()
    wq_d = nc.dram_tensor("wq", [HID, D], F16, kind="ExternalInput").ap()
    wk_d = nc.dram_tensor("wk", [HID, D], F16, kind="ExternalInput").ap()
    wv_d = nc.dram_tensor("wv", [HID, D], F16, kind="ExternalInput").ap()
    if with_qk_bias:
        bq_d = nc.dram_tensor("bq", [D], F32, kind="ExternalInput").ap()
        bk_d = nc.dram_tensor("bk", [D], F32, kind="ExternalInput").ap()
    idf_d = nc.dram_tensor("idf", [128, 128], F32, kind="ExternalInput").ap()
    out_d = nc.dram_tensor("out", [S, D], F32, kind="ExternalOutput").ap()

    ExpF = mybir.ActivationFunctionType.Exp

    qT_r = qT_d.rearrange("(o p) s -> p o s", p=128)   # [128, HCH, S]
    kT_r = kT_d.rearrange("(o p) s -> p o s", p=128)
    vT_r = vT_d.rearrange("(o p) s -> p o s", p=128)

    def _body(tc):
        with tc.tile_pool(name="const", bufs=1) as const:
            w_q = const.tile([128, HCH, D], F16, name="w_q")
            w_k = const.tile([128, HCH, D], F16, name="w_k")
            w_v = const.tile([128, HCH, D], F16, name="w_v")
            idf = const.tile([128, 128], F32, name="idf")
            idf16 = const.tile([128, 128], F16, name="idf16")

            # masksb pair-major: [p, pair, slot, q] with k-tile = 8g+4s+i,
            # pair index 4g+i, slot s.
            masksb = const.tile([128, NPAIR, 2, S], FP8, name="masksb")
            qh = const.tile([128, HCH, S], F16, name="qh")
            kh = const.tile([128, HCH, S], F16, name="kh")
            vh = const.tile([128, HCH, S], F16, name="vh")

            # ---- DMA issue order == consumption order -------------------
            # scalar (ACT): first two q chunks only, then ACT is all exps.
            for c in range(2):
                cs = slice(c * NQ, (c + 1) * NQ)
                for h in range(0, HCH, 2):
                    nc.scalar.dma_start(qh[:, h:h + 2, cs],
                                        qT_r[:, h:h + 2, cs])

            # gpsimd queue (empirically the fastest): all of k
            # (col-half major), then masks (pair-group major).
            cs0 = slice(0, 1024)
            cs1 = slice(1024, 2048)
            for csl in (cs0, cs1):
                for h in range(0, HCH, 2):
                    nc.gpsimd.dma_start(kh[:, h:h + 2, csl],
                                        kT_r[:, h:h + 2, csl])
            for g in range(2):
                for s_ in range(2):
                    r0 = (8 * g + 4 * s_) * 128
                    nc.gpsimd.dma_start(
                        masksb[:, 4 * g:4 * (g + 1), s_, :],
                        maskT_d[r0:r0 + 4 * 128, :].rearrange(
                            "(i p) q -> p i q", p=128))

            # sync: weights, q col-half1, v (h0-3), outs at the end.
            nc.sync.dma_start(w_q, wq_d.rearrange("(o p) d -> p o d", p=128))
            nc.sync.dma_start(w_k, wk_d.rearrange("(o p) d -> p o d", p=128))
            nc.sync.dma_start(w_v, wv_d.rearrange("(o p) d -> p o d", p=128))
            nc.sync.dma_start(idf, idf_d)
            if with_qk_bias:
                b_q = const.tile([128, 1], F32, name="b_q")
                b_k = const.tile([128, 1], F32, name="b_k")
                nc.sync.dma_start(b_q[0:D, :], bq_d.unsqueeze(1))
                nc.sync.dma_start(b_q[64:64 + D, :], bq_d.unsqueeze(1))
                nc.sync.dma_start(b_k[0:D, :], bk_d.unsqueeze(1))
                nc.sync.dma_start(b_k[64:64 + D, :], bk_d.unsqueeze(1))
            else:
                b_q = b_k = None
            for h in range(0, HCH, 2):
                nc.sync.dma_start(qh[:, h:h + 2, cs1], qT_r[:, h:h + 2, cs1])
            for h in range(0, 4, 2):
                nc.sync.dma_start(vh[:, h:h + 2, :], vT_r[:, h:h + 2, :])

            # scalar also takes v h4-7 after the q chunks (2 triggers,
            # issued before the exp stream owns ACT).
            for h in range(4, HCH, 2):
                nc.scalar.dma_start(vh[:, h:h + 2, :], vT_r[:, h:h + 2, :])

            nc.vector.tensor_copy(idf16, idf)

            QT = const.tile([128, S], F16, name="QT")
            KT = const.tile([128, S], F16, name="KT")
            VT = const.tile([128, S], F16, name="VT")
            Vt = const.tile([128, KT_TILES, D + 1], F16, name="Vt")
            nc.vector.memset(Vt[:, :, D:D + 1], 1.0)

            with tc.tile_pool(name="stp", bufs=2, space="PSUM") as stp, \
                 tc.tile_pool(name="ntp", bufs=2, space="PSUM") as ntp, \
                 tc.tile_pool(name="ptp", bufs=26) as ptp, \
                 tc.tile_pool(name="nsb", bufs=2) as nsb:

                def q_proj(c):
                    # both column groups get the same data (QT rows 64-127
                    # duplicate rows 0-63 for score row packing); the two
                    # groups run concurrently so the dup is free.
                    cs = slice(c * NQ, (c + 1) * NQ)
                    prja = stp.tile([128, NQ], F32, name="prja", tag="prj",
                                    bufs=2)
                    prjb = stp.tile([128, NQ], F32, name="prjb", tag="prj",
                                    bufs=2)
                    for h in range(HCH):
                        st_, sp_ = (h == 0), (h == HCH - 1)
                        nc.tensor.matmul(
                            prja[0:D, :], lhsT=w_q[:, h, :],
                            rhs=qh[:, h, cs], start=st_, stop=sp_)
                        nc.tensor.matmul(
                            prjb[64:64 + D, :], lhsT=w_q[:, h, :],
                            rhs=qh[:, h, cs], start=st_, stop=sp_)
                    nc.vector.tensor_copy(QT[0:D, cs], prja[0:D, :])
                    nc.vector.tensor_copy(QT[64:128, cs], prjb[64:128, :])
                    if b_q is not None:
                        nc.vector.tensor_scalar_add(QT[:, cs], QT[:, cs], b_q)

                def kv_proj_cp(hid_t, w_t, b_t, dest, cp, eng):
                    # one column-packed pair of 512-chunks: ca -> rows 0-63,
                    # cb -> rows 64-127.
                    ca = slice((2 * cp) * 512, (2 * cp + 1) * 512)
                    cb = slice((2 * cp + 1) * 512, (2 * cp + 2) * 512)
                    prja = stp.tile([128, 512], F32, name="prja", tag="prj",
                                    bufs=2)
                    prjb = stp.tile([128, 512], F32, name="prjb", tag="prj",
                                    bufs=2)
                    for h in range(HCH):
                        st_, sp_ = (h == 0), (h == HCH - 1)
                        nc.tensor.matmul(
                            prja[0:D, :], lhsT=w_t[:, h, :],
                            rhs=hid_t[:, h, ca], start=st_, stop=sp_)
                        nc.tensor.matmul(
                            prjb[64:64 + D, :], lhsT=w_t[:, h, :],
                            rhs=hid_t[:, h, cb], start=st_, stop=sp_)
                    eng.tensor_copy(dest[0:D, ca], prja[0:D, :])
                    eng.tensor_copy(dest[64:128, cb], prjb[64:128, :])
                    if b_t is not None:
                        eng.tensor_scalar_add(
                            dest[0:D, ca], dest[0:D, ca], b_t[0:D, :])
                        eng.tensor_scalar_add(
                            dest[64:64 + D, cb], dest[64:64 + D, cb],
                            b_t[64:64 + D, :])

                def v_finish():
                    # V^T -> Vt[p, kt, D+1]; odd 512-chunks of VT live on
                    # rows 64-127 (column packing).
                    for kt in range(KT_TILES):
                        rb = 0 if (kt // 4) % 2 == 0 else 64
                        vtr = ntp.tile([128, D], F16, name="vtr", tag="tr")
                        nc.tensor.transpose(
                            vtr, VT[rb:rb + D, kt * 128:(kt + 1) * 128],
                            idf16[rb:rb + D, rb:rb + D])
                        nc.vector.tensor_copy(Vt[:, kt, :D], vtr)

                def sc_exp(qc, p):
                    # row-packed fp16 score pair (kta rows 0-63, ktb rows
                    # 64-127), exp -> fp16.
                    q0 = qc * NQ
                    qsl = slice(q0, q0 + NQ)
                    g, i = divmod(p, 4)
                    kta, ktb = 8 * g + i, 8 * g + i + 4
                    sa = slice(kta * 128, kta * 128 + 128)
                    sb = slice(ktb * 128, ktb * 128 + 128)
                    st = stp.tile([128, 2, NQ], F32, name="st", tag="st")
                    nc.tensor.matmul(
                        st[:, 0, :], lhsT=KT[0:D, sa], rhs=QT[0:D, qsl],
                        start=True, stop=True)
                    nc.tensor.matmul(
                        st[:, 1, :], lhsT=KT[64:64 + D, sb],
                        rhs=QT[64:64 + D, qsl],
                        start=True, stop=True)
                    pt = ptp.tile([128, 2, NQ], F16, name="pt", tag="pt")
                    nc.scalar.activation(pt, st, ExpF)
                    return pt

                def mask_mul(qc, p, pt):
                    q0 = qc * NQ
                    qsl = slice(q0, q0 + NQ)
                    eng = nc.gpsimd if (qc * NPAIR + p) % 4 == 3 else nc.vector
                    eng.tensor_mul(pt, pt, masksb[:, p, :, qsl])

                def av(outT, p, pt, first, last):
                    g, i = divmod(p, 4)
                    kta, ktb = 8 * g + i, 8 * g + i + 4
                    nc.tensor.matmul(
                        outT, lhsT=Vt[:, kta, :], rhs=pt[:, 0, :],
                        start=first, stop=False)
                    nc.tensor.matmul(
                        outT, lhsT=Vt[:, ktb, :], rhs=pt[:, 1, :],
                        start=False, stop=last)

                def norm(qc, outT):
                    q0 = qc * NQ
                    outT_sb = nsb.tile([D + 1, NQ], F32, name="outT_sb",
                                       tag="outT_sb")
                    nc.vector.tensor_copy(outT_sb, outT)
                    o_big = nsb.tile([128, NQ // 128, D], F32, name="o_big",
                                     tag="o_big")
                    for i in range(NQ // 128):
                        tr = ntp.tile([128, D + 1], F32, name="tr", tag="tr")
                        nc.tensor.transpose(
                            tr, outT_sb[:, i * 128:(i + 1) * 128],
                            idf[:D + 1, :D + 1])
                        tr_sb = nsb.tile([128, D + 1], F32, name="tr_sb",
                                         tag="tr_sb")
                        nc.vector.tensor_copy(tr_sb, tr)
                        nc.vector.reciprocal(tr_sb[:, D:D + 1],
                                             tr_sb[:, D:D + 1])
                        nc.vector.tensor_scalar_mul(
                            o_big[:, i, :], tr_sb[:, :D], tr_sb[:, D:D + 1])
                    nc.sync.dma_start(
                        out_d[q0:q0 + NQ, :].rearrange("(t p) d -> p t d",
                                                       p=128), o_big)

                # ---- staged emission (PE stream order == data arrival) --
                # wave 0 = pairs 0-3 (k-tiles 0-7: K chunk-pair 0, mask
                # g=0 blocks); wave 1 = pairs 4-7 (k-tiles 8-15).
                pts = {}
                kv_proj_cp(kh, w_k, b_k, KT, 0, nc.vector)
                for qc in range(QCH):
                    q_proj(qc)
                    for p in range(4):
                        pts[(qc, p)] = sc_exp(qc, p)
                        mask_mul(qc, p, pts[(qc, p)])
                kv_proj_cp(vh, w_v, None, VT, 0, nc.vector)
                kv_proj_cp(vh, w_v, None, VT, 1, nc.vector)
                v_finish()
                kv_proj_cp(kh, w_k, b_k, KT, 1, nc.vector)
                for qc in range(QCH):
                    for p in range(4, NPAIR):
                        pts[(qc, p)] = sc_exp(qc, p)
                        mask_mul(qc, p, pts[(qc, p)])
                    outT = stp.tile([D + 1, NQ], F32, name="outT",
                                    tag="prj", bufs=2)
                    for p in range(NPAIR):
                        av(outT, p, pts[(qc, p)], p == 0, p == NPAIR - 1)
                    norm(qc, outT)

    with tile.TileContext(nc) as tc:
        _body(tc)

    nc.compile()
    return nc


def _prep_inputs(q_hidden_inputs, k_hidden_inputs, v_hidden_inputs, mask,
                 Wq, bq, Wk, bk, Wv, bv):
    scale = np.float32(1.0 / np.sqrt(np.float32(D)))
    wq = (np.asarray(Wq, np.float32) * scale).astype(np.float16)
    wk = np.asarray(Wk, np.float32).astype(np.float16)
    wv = np.asarray(Wv, np.float32).astype(np.float16)
    bqs = np.asarray(bq, np.float32) * scale
    bks = np.asarray(bk, np.float32)
    with_qk_bias = bool(np.any(bqs != 0) or np.any(bks != 0))
    idf = np.eye(128, dtype=np.float32)

    q = np.asarray(q_hidden_inputs, np.float32)
    k = np.asarray(k_hidden_inputs, np.float32)
    v = np.asarray(v_hidden_inputs, np.float32)
    m = np.asarray(mask)

    import ml_dtypes
    FP8_NP = ml_dtypes.float8_e4m3

    in_maps = []
    for b in range(B):
        im = {
            "qT": np.ascontiguousarray(q[b].T).astype(np.float16),
            "kT": np.ascontiguousarray(k[b].T).astype(np.float16),
            "vT": np.ascontiguousarray(v[b].T).astype(np.float16),
            "maskT": (np.ascontiguousarray(m[b].T) != 0).astype(
                np.float32).astype(FP8_NP),
            "wq": wq, "wk": wk, "wv": wv,
            "idf": idf,
        }
        if with_qk_bias:
            im["bq"] = bqs
            im["bk"] = bks
        in_maps.append(im)
    return in_maps, with_qk_bias


def kernel(q_hidden_inputs, k_hidden_inputs, v_hidden_inputs, mask,
           Wq, bq, Wk, bk, Wv, bv, trace=False):
    global LAST_EXEC_TIME_NS
    in_maps, with_qk_bias = _prep_inputs(
        q_hidden_inputs, k_hidden_inputs, v_hidden_inputs,
        mask, Wq, bq, Wk, bk, Wv, bv)
    key = ("nc", with_qk_bias)
    if key not in _CACHED:
        _CACHED[key] = _build_program(with_qk_bias)
    nc = _CACHED[key]

    res = run_bass_kernel_spmd(nc, in_maps, list(range(NCORES)), trace=trace)
    LAST_EXEC_TIME_NS = res.exec_time_ns
    out = np.stack([res.results[b]["out"] for b in range(B)], axis=0)
    # bv folds into the output exactly: softmax rows sum to 1, so
    # attn @ (V + 1 bv^T) = attn @ V + bv.
    out = out + np.asarray(bv, np.float32)[None, None, :]
    return out


# revision 16
# speedup vs baseline: 1.1997x; 1.0482x over previous
"""Trainium2 Bass kernel for masked single-head attention.

Reference computation (per batch b):
    Q = q_hidden[b] @ Wq + bq            # [S, D]
    K = k_hidden[b] @ Wk + bk            # [S, D]
    V = v_hidden[b] @ Wv + bv            # [S, D]
    S_qk = (Q @ K.T) / sqrt(D)           # [S, S]
    S_qk = where(mask[b]==0, -1e9, S_qk)
    out[b] = softmax(S_qk, -1) @ V       # [S, D]

Sharding: data-parallel over batch, one batch per NeuronCore (B == 8 cores).
No collectives.

Device-side dataflow (per core, S=2048, HID=1024, D=64):
  - host ships transposed hiddens qT/kT/vT [HID, S] fp16, the mask
    transposed as {0,1} fp8, Wq pre-scaled by 1/sqrt(D).  (fp8 hiddens /
    probabilities were tried and fail the 2e-2 gate: random-sign sums do
    not shrink relative quantization error, and the scale-relative MAX
    metric sits ~4.5 sigma out; measured 2-9e-2 per fp8 tensor.)
  - projections: fp16 matmuls column-packed into PSUM partition groups
    0-63 / 64-127 (concurrent, so the pair costs one stream).  Q is
    projected into both groups (score row packing needs QT duplicated).
  - scores: fp16 QK^T row-packed pairs (kta on array rows 0-63, ktb=kta+4
    on rows 64-127) into one [128, 2, 512] PSUM tile; exp on ScalarE
    emits fp16; the mask is applied post-exp as a multiply on VectorE
    (masked lanes become exactly 0, no -1e9 offset needed since
    unmasked scores are < 3).
  - out^T[65, q] accumulates over k tiles: lhsT=[V_kt|1] [128, 65];
    row 64 is the softmax denominator.
  - norm: PE-transpose [65,128] slices back to [128,65], reciprocal of
    the per-partition denominator column (cheap: [128,1] layout),
    multiply on GpSimd, DMA out [q, 64].
  - engine split: ScalarE = exps only (plus early q DMA triggers);
    VectorE = K copies + mask multiplies + reciprocals; GpSimd = Q/V
    copies, norm copies/muls, mask+k DMA triggers; SyncE = bulk DMA.
"""

import numpy as np

import concourse.bass as bass
import concourse.tile as tile
from concourse import bacc
from concourse import mybir
from concourse.bass_utils import run_bass_kernel_spmd

B, S, HID, D = 8, 2048, 1024, 64
NCORES = 8
HCH = HID // 128          # 8 hidden chunks
KT_TILES = S // 128       # 16 k tiles
NPAIR = KT_TILES // 2     # 8 k-tile pairs (kta, kta+4)
NQ = 512                  # q chunk width for the attention inner loop
QCH = S // NQ             # 4

F32 = mybir.dt.float32
F16 = mybir.dt.float16
FP8 = mybir.dt.float8e4

LAST_EXEC_TIME_NS = None
_CACHED = {}


def _build_program(with_qk_bias=False):
    nc = bacc.Bacc("TRN2", target_bir_lowering=False, debug=False,
                   num_swdge_queues=4)

    qT_d = nc.dram_tensor("qT", [HID, S], F16, kind="ExternalInput").ap()
    kT_d = nc.dram_tensor("kT", [HID, S], F16, kind="ExternalInput").ap()
    vT_d = nc.dram_tensor("vT", [HID, S], F16, kind="ExternalInput").ap()
    maskT_d = nc.dram_tensor("maskT", [S, S], FP8, kind="ExternalInput").ap()
    wq_d = nc.dram_tensor("wq", [HID, D], F16, kind="ExternalInput").ap()
    wk_d = nc.dram_tensor("wk", [HID, D], F16, kind="ExternalInput").ap()
    wv_d = nc.dram_tensor("wv", [HID, D], F16, kind="ExternalInput").ap()
    if with_qk_bias:
        bq_d = nc.dram_tensor("bq", [D], F32, kind="ExternalInput").ap()
        bk_d = nc.dram_tensor("bk", [D], F32, kind="ExternalInput").ap()
    idf_d = nc.dram_tensor("idf", [128, 128], F32, kind="ExternalInput").ap()
    out_d = nc.dram_tensor("out", [S, D], F32, kind="ExternalOutput").ap()

    ExpF = mybir.ActivationFunctionType.Exp

    qT_r = qT_d.rearrange("(o p) s -> p o s", p=128)   # [128, HCH, S]
    kT_r = kT_d.rearrange("(o p) s -> p o s", p=128)
    vT_r = vT_d.rearrange("(o p) s -> p o s", p=128)

    def _body(tc):
        with tc.tile_pool(name="const", bufs=1) as const:
            w_q = const.tile([128, HCH, D], F16, name="w_q")
            w_k = const.tile([128, HCH, D], F16, name="w_k")
            w_v = const.tile([128, HCH, D], F16, name="w_v")
            idf = const.tile([128, 128], F32, name="idf")
            idf16 = const.tile([128, 128], F16, name="idf16")

            # masksb pair-major: [p, pair, slot, q] with k-tile = 8g+4s+i,
            # pair index 4g+i, slot s.
            masksb = const.tile([128, NPAIR, 2, S], FP8, name="masksb")
            qh = const.tile([128, HCH, S], F16, name="qh")
            kh = const.tile([128, HCH, S], F16, name="kh")
            vh = const.tile([128, HCH, S], F16, name="vh")

            # ---- DMA issue order == consumption order -------------------
            # gpsimd queue (empirically the fastest, ~60%% of bytes), in
            # strict consumption order: k col-half0, q c1, q col-half1,
            # masks g0, v h4-7, masks g1.
            cs0 = slice(0, 1024)
            cs1 = slice(1024, 2048)
            c1s = slice(NQ, 2 * NQ)

            def mask_dma(g, s_):
                r0 = (8 * g + 4 * s_) * 128
                nc.gpsimd.dma_start(
                    masksb[:, 4 * g:4 * (g + 1), s_, :],
                    maskT_d[r0:r0 + 4 * 128, :].rearrange(
                        "(i p) q -> p i q", p=128))

            for h in range(0, HCH, 2):
                nc.gpsimd.dma_start(kh[:, h:h + 2, cs0], kT_r[:, h:h + 2, cs0])
            for h in range(0, HCH, 2):
                nc.gpsimd.dma_start(qh[:, h:h + 2, c1s], qT_r[:, h:h + 2, c1s])
            for h in range(0, HCH, 2):
                nc.gpsimd.dma_start(qh[:, h:h + 2, cs1], qT_r[:, h:h + 2, cs1])
            mask_dma(0, 0)
            mask_dma(0, 1)
            for h in range(4, HCH, 2):
                nc.gpsimd.dma_start(vh[:, h:h + 2, :], vT_r[:, h:h + 2, :])
            mask_dma(1, 0)
            mask_dma(1, 1)

            # sync: weights, v h0-3, k col-half1, outs at the end.
            nc.sync.dma_start(w_q, wq_d.rearrange("(o p) d -> p o d", p=128))
            nc.sync.dma_start(w_k, wk_d.rearrange("(o p) d -> p o d", p=128))
            nc.sync.dma_start(w_v, wv_d.rearrange("(o p) d -> p o d", p=128))
            nc.sync.dma_start(idf, idf_d)
            if with_qk_bias:
                b_q = const.tile([128, 1], F32, name="b_q")
                b_k = const.tile([128, 1], F32, name="b_k")
                nc.sync.dma_start(b_q[0:D, :], bq_d.unsqueeze(1))
                nc.sync.dma_start(b_q[64:64 + D, :], bq_d.unsqueeze(1))
                nc.sync.dma_start(b_k[0:D, :], bk_d.unsqueeze(1))
                nc.sync.dma_start(b_k[64:64 + D, :], bk_d.unsqueeze(1))
            else:
                b_q = b_k = None
            for h in range(0, 4, 2):
                nc.sync.dma_start(vh[:, h:h + 2, :], vT_r[:, h:h + 2, :])
            for h in range(0, HCH, 2):
                nc.sync.dma_start(kh[:, h:h + 2, cs1], kT_r[:, h:h + 2, cs1])

            # scalar: only q c0 (4 small triggers), ACT is free from ~3us.
            c0s = slice(0, NQ)
            for h in range(0, HCH, 2):
                nc.scalar.dma_start(qh[:, h:h + 2, c0s],
                                    qT_r[:, h:h + 2, c0s])

            nc.vector.tensor_copy(idf16, idf)

            QT = const.tile([128, S], F16, name="QT")
            KT = const.tile([128, S], F16, name="KT")
            VT = const.tile([128, S], F16, name="VT")
            Vt = const.tile([128, KT_TILES, D + 1], F16, name="Vt")
            nc.vector.memset(Vt[:, :, D:D + 1], 1.0)

            with tc.tile_pool(name="stp", bufs=2, space="PSUM") as stp, \
                 tc.tile_pool(name="ntp", bufs=2, space="PSUM") as ntp, \
                 tc.tile_pool(name="ptp", bufs=26) as ptp, \
                 tc.tile_pool(name="nsb", bufs=2) as nsb:

                def q_proj(c):
                    # both column groups get the same data (QT rows 64-127
                    # duplicate rows 0-63 for score row packing); the two
                    # groups run concurrently so the dup is free.
                    cs = slice(c * NQ, (c + 1) * NQ)
                    prja = stp.tile([128, NQ], F32, name="prja", tag="prj",
                                    bufs=2)
                    prjb = stp.tile([128, NQ], F32, name="prjb", tag="prj",
                                    bufs=2)
                    for h in range(HCH):
                        st_, sp_ = (h == 0), (h == HCH - 1)
                        nc.tensor.matmul(
                            prja[0:D, :], lhsT=w_q[:, h, :],
                            rhs=qh[:, h, cs], start=st_, stop=sp_)
                        nc.tensor.matmul(
                            prjb[64:64 + D, :], lhsT=w_q[:, h, :],
                            rhs=qh[:, h, cs], start=st_, stop=sp_)
                    nc.vector.tensor_copy(QT[0:D, cs], prja[0:D, :])
                    nc.vector.tensor_copy(QT[64:128, cs], prjb[64:128, :])
                    if b_q is not None:
                        nc.vector.tensor_scalar_add(QT[:, cs], QT[:, cs], b_q)

                def kv_proj_cp(hid_t, w_t, b_t, dest, cp, eng):
                    # one column-packed pair of 512-chunks: ca -> rows 0-63,
                    # cb -> rows 64-127.
                    ca = slice((2 * cp) * 512, (2 * cp + 1) * 512)
                    cb = slice((2 * cp + 1) * 512, (2 * cp + 2) * 512)
                    prja = stp.tile([128, 512], F32, name="prja", tag="prj",
                                    bufs=2)
                    prjb = stp.tile([128, 512], F32, name="prjb", tag="prj",
                                    bufs=2)
                    for h in range(HCH):
                        st_, sp_ = (h == 0), (h == HCH - 1)
                        nc.tensor.matmul(
                            prja[0:D, :], lhsT=w_t[:, h, :],
                            rhs=hid_t[:, h, ca], start=st_, stop=sp_)
                        nc.tensor.matmul(
                            prjb[64:64 + D, :], lhsT=w_t[:, h, :],
                            rhs=hid_t[:, h, cb], start=st_, stop=sp_)
                    eng.tensor_copy(dest[0:D, ca], prja[0:D, :])
                    eng.tensor_copy(dest[64:128, cb], prjb[64:128, :])
                    if b_t is not None:
                        eng.tensor_scalar_add(
                            dest[0:D, ca], dest[0:D, ca], b_t[0:D, :])
                        eng.tensor_scalar_add(
                            dest[64:64 + D, cb], dest[64:64 + D, cb],
                            b_t[64:64 + D, :])

                def v_finish():
                    # V^T -> Vt[p, kt, D+1]; odd 512-chunks of VT live on
                    # rows 64-127 (column packing).
                    for kt in range(KT_TILES):
                        rb = 0 if (kt // 4) % 2 == 0 else 64
                        vtr = ntp.tile([128, D], F16, name="vtr", tag="tr")
                        nc.tensor.transpose(
                            vtr, VT[rb:rb + D, kt * 128:(kt + 1) * 128],
                            idf16[rb:rb + D, rb:rb + D])
                        nc.vector.tensor_copy(Vt[:, kt, :D], vtr)

                def sc_exp(qc, p):
                    # row-packed fp16 score pair (kta rows 0-63, ktb rows
                    # 64-127), exp -> fp16.
                    q0 = qc * NQ
                    qsl = slice(q0, q0 + NQ)
                    g, i = divmod(p, 4)
                    kta, ktb = 8 * g + i, 8 * g + i + 4
                    sa = slice(kta * 128, kta * 128 + 128)
                    sb = slice(ktb * 128, ktb * 128 + 128)
                    st = stp.tile([128, 2, NQ], F32, name="st", tag="st")
                    nc.tensor.matmul(
                        st[:, 0, :], lhsT=KT[0:D, sa], rhs=QT[0:D, qsl],
                        start=True, stop=True)
                    nc.tensor.matmul(
                        st[:, 1, :], lhsT=KT[64:64 + D, sb],
                        rhs=QT[64:64 + D, qsl],
                        start=True, stop=True)
                    pt = ptp.tile([128, 2, NQ], F16, name="pt", tag="pt")
                    nc.scalar.activation(pt, st, ExpF)
                    return pt

                def mask_mul(qc, p, pt):
                    q0 = qc * NQ
                    qsl = slice(q0, q0 + NQ)
                    eng = nc.gpsimd if (qc * NPAIR + p) % 4 == 3 else nc.vector
                    eng.tensor_mul(pt, pt, masksb[:, p, :, qsl])

                def av(outT, p, pt, first, last):
                    g, i = divmod(p, 4)
                    kta, ktb = 8 * g + i, 8 * g + i + 4
                    nc.tensor.matmul(
                        outT, lhsT=Vt[:, kta, :], rhs=pt[:, 0, :],
                        start=first, stop=False)
                    nc.tensor.matmul(
                        outT, lhsT=Vt[:, ktb, :], rhs=pt[:, 1, :],
                        start=False, stop=last)

                def norm(qc, outT):
                    q0 = qc * NQ
                    outT_sb = nsb.tile([D + 1, NQ], F32, name="outT_sb",
                                       tag="outT_sb")
                    nc.vector.tensor_copy(outT_sb, outT)
                    o_big = nsb.tile([128, NQ // 128, D], F32, name="o_big",
                                     tag="o_big")
                    for i in range(NQ // 128):
                        tr = ntp.tile([128, D + 1], F32, name="tr", tag="tr")
                        nc.tensor.transpose(
                            tr, outT_sb[:, i * 128:(i + 1) * 128],
                            idf[:D + 1, :D + 1])
                        nc.vector.reciprocal(tr[:, D:D + 1],
                                             tr[:, D:D + 1])
                        nc.vector.tensor_scalar_mul(
                            o_big[:, i, :], tr[:, :D], tr[:, D:D + 1])
                    nc.sync.dma_start(
                        out_d[q0:q0 + NQ, :].rearrange("(t p) d -> p t d",
                                                       p=128), o_big)

                # ---- staged emission (PE stream order == data arrival) --
                # wave 0 = pairs 0-3 (k-tiles 0-7: K chunk-pair 0, mask
                # g=0 blocks); wave 1 = pairs 4-7 (k-tiles 8-15).
                pts = {}
                kv_proj_cp(kh, w_k, b_k, KT, 0, nc.vector)
                for qc in range(QCH):
                    q_proj(qc)
                    for p in range(4):
                        pts[(qc, p)] = sc_exp(qc, p)
                        mask_mul(qc, p, pts[(qc, p)])
                kv_proj_cp(vh, w_v, None, VT, 0, nc.vector)
                kv_proj_cp(vh, w_v, None, VT, 1, nc.vector)
                v_finish()
                kv_proj_cp(kh, w_k, b_k, KT, 1, nc.vector)
                for qc in range(QCH):
                    for p in range(4, NPAIR):
                        pts[(qc, p)] = sc_exp(qc, p)
                        mask_mul(qc, p, pts[(qc, p)])
                    outT = stp.tile([D + 1, NQ], F32, name="outT",
                                    tag="prj", bufs=2)
                    for p in range(NPAIR):
                        av(outT, p, pts[(qc, p)], p == 0, p == NPAIR - 1)
                    norm(qc, outT)

    with tile.TileContext(nc) as tc:
        _body(tc)

    nc.compile()
    return nc


def _prep_inputs(q_hidden_inputs, k_hidden_inputs, v_hidden_inputs, mask,
                 Wq, bq, Wk, bk, Wv, bv):
    scale = np.float32(1.0 / np.sqrt(np.float32(D)))
    wq = (np.asarray(Wq, np.float32) * scale).astype(np.float16)
    wk = np.asarray(Wk, np.float32).astype(np.float16)
    wv = np.asarray(Wv, np.float32).astype(np.float16)
    bqs = np.asarray(bq, np.float32) * scale
    bks = np.asarray(bk, np.float32)
    with_qk_bias = bool(np.any(bqs != 0) or np.any(bks != 0))
    idf = np.eye(128, dtype=np.float32)

    q = np.asarray(q_hidden_inputs, np.float32)
    k = np.asarray(k_hidden_inputs, np.float32)
    v = np.asarray(v_hidden_inputs, np.float32)
    m = np.asarray(mask)

    import ml_dtypes
    FP8_NP = ml_dtypes.float8_e4m3

    in_maps = []
    for b in range(B):
        im = {
            "qT": np.ascontiguousarray(q[b].T).astype(np.float16),
            "kT": np.ascontiguousarray(k[b].T).astype(np.float16),
            "vT": np.ascontiguousarray(v[b].T).astype(np.float16),
            "maskT": (np.ascontiguousarray(m[b].T) != 0).astype(
                np.float32).astype(FP8_NP),
            "wq": wq, "wk": wk, "wv": wv,
            "idf": idf,
        }
        if with_qk_bias:
            im["bq"] = bqs
            im["bk"] = bks
        in_maps.append(im)
    return in_maps, with_qk_bias


def kernel(q_hidden_inputs, k_hidden_inputs, v_hidden_inputs, mask,
           Wq, bq, Wk, bk, Wv, bv, trace=False):
    global LAST_EXEC_TIME_NS
    in_maps, with_qk_bias = _prep_inputs(
        q_hidden_inputs, k_hidden_inputs, v_hidden_inputs,
        mask, Wq, bq, Wk, bk, Wv, bv)
    key = ("nc", with_qk_bias)
    if key not in _CACHED:
        _CACHED[key] = _build_program(with_qk_bias)
    nc = _CACHED[key]

    res = run_bass_kernel_spmd(nc, in_maps, list(range(NCORES)), trace=trace)
    LAST_EXEC_TIME_NS = res.exec_time_ns
    out = np.stack([res.results[b]["out"] for b in range(B)], axis=0)
    # bv folds into the output exactly: softmax rows sum to 1, so
    # attn @ (V + 1 bv^T) = attn @ V + bv.
    out = out + np.asarray(bv, np.float32)[None, None, :]
    return out
